# revision 47
# baseline (speedup 1.0000x reference)
"""Cosine-similarity self-attention (Cos_Attn) on 8 Trainium2 NeuronCores.

Reference math (x: [C=512, W=64, H=64] fp32, N = W*H = 4096):
    q = x.reshape(C, N).T                  # [N, C]
    energy = q @ q.T                       # [N, N]
    cos    = energy / (|q_i| |q_j|)
    out    = softmax(cos, axis=-1)[None]   # [1, N, N]

v23 design - host-normalized fp8, query-major layout, PE warm-up,
ACT-accumulated row sums, arrival-interleaved gapless head.

Host pre-normalizes the columns of x to unit L2 norm before the fp8
quantize, so on device cosine == dot product of fp8 unit vectors: no
Grams, no rsqrt chains, one ACT table load (pulled to t=0 by a dummy
exp). Inputs are host-permuted so every input DMA descriptor is a 4 KB
contiguous run.

Per core (own 512 query rows x all 4096 keys):
  - queries on PSUM partitions, keys free; output needs no transpose.
  - PE warm-up: a few dummy bf16 matmuls run during the input DMA so
    the Tensor engine leaves its cold pstate and real matmuls overlap
    their LDWEIGHTS immediately.
  - energy: per (128-query block, 2048-key half): 8 fp8 DoubleRow
    matmuls (K=256) into a 4-bank PSUM tile [128, 4, 512], double
    buffered (PE fills one while ACT drains the other).
  - head: query blocks 0-2 process keys 0:2048 at chunk (1024-key)
    granularity, interleaved in DMA-arrival order: the first exp waits
    only for the first key pair (~14 us vs ~18), and blocks 1-2's
    pair-0/1 chunks fill the ACT waits while later pairs are still in
    flight (pair0 serves every block) - the exp stream is gapless.
    Second halves and block 3 use 2048-key halves (lower ACT
    per-instruction overhead).
  - exp: ONE activation per half ([128, 2048] f32 PSUM -> bf16 SBUF,
    scale=1/cq^2) with accum_out producing the row-sum for ~180 ns;
    softmax denominators never touch the slow 1x-mode DVE reduce. ACT
    is the bottleneck engine (~2.2 us x 8 stream). DVE carries no exp
    work, so the tile scheduler interleaves the softmax scales (and
    their 1 MB out-DMAs) into the stream instead of deferring them.
  - tail: reciprocal_approx_fast, all-bf16 per-partition scale (2x
    mode); early blocks' 1 MB out-DMAs overlap later blocks' compute;
    the last block's scale+DMA is split across the two free queues.
"""

import numpy as np

_NCORES = 8
_P = 128

# set by the test harness only; the grading path keeps these defaults
TRACE = False
TRACE_CORES = None
LAST_RESULT = None

_built = None  # (nc, C, N)

_CQ = 16.0     # host fp8 quantize scale for the normalized columns
_NWARM = 8     # PE pstate warm-up matmuls


def _build(C, N, QB):
    """Single-NEFF Bass/Tile program (SPMD: identical on all cores).

    Inputs:  x8 [128, C/128 * N]  fp8e4, host-permuted pair-major:
                 [p, pair(4), ko(4), 1024] with c = ko*128 + p
             xq [128, C/128 * QB] fp8e4, host-permuted: [p, ko(4), QB]
    Output:  out [QB, N] bf16 = this core's softmax rows.
    """
    from contextlib import ExitStack

    import concourse.tile as tile
    from concourse import bacc, mybir

    f32 = mybir.dt.float32
    bf16 = mybir.dt.bfloat16
    fp8 = mybir.dt.float8e4
    AF = mybir.ActivationFunctionType
    DR = mybir.MatmulPerfMode.DoubleRow

    P = _P
    KO = C // P              # contraction subtiles (4)
    NP = N // 1024           # key chunk pairs (4)
    QBLK = QB // P           # query blocks per core (4)
    ESC = 1.0 / (_CQ * _CQ)  # exp input scale: cos = energy / cq^2

    nc = bacc.Bacc("TRN2", target_bir_lowering=False, debug=False)
    x8_d = nc.dram_tensor("x8", [P, KO * N], fp8, kind="ExternalInput")
    xq_d = nc.dram_tensor("xq", [P, KO * QB], fp8, kind="ExternalInput")
    out_d = nc.dram_tensor("out", [QB, N], bf16, kind="ExternalOutput")

    x8_r = x8_d.ap().rearrange("p (c k x) -> p c k x", c=NP, k=KO)
    xq_r = xq_d.ap().rearrange("p (k x) -> p k x", k=KO)
    out_r = out_d.ap().rearrange("(qb p) (nk x) -> p qb nk x", p=P, x=512)

    with tile.TileContext(nc) as tc, ExitStack() as ctx:
        persist = ctx.enter_context(tc.tile_pool(name="persist", bufs=1))
        temps = ctx.enter_context(tc.tile_pool(name="temps", bufs=2))
        psum = ctx.enter_context(tc.tile_pool(name="psum", bufs=2, space="PSUM"))

        x8_sb = persist.tile([P, NP, KO, 1024], fp8)   # all keys, pair-major
        xq_sb = persist.tile([P, KO, QB], fp8)         # own query cols
        e_sb = persist.tile([P, QBLK, 2, 2048], bf16)  # exp(cos) rows
        rsum = persist.tile([P, QBLK, 2], f32)         # half row-sums
        rs = persist.tile([P, QBLK], f32)              # row sums
        rr = persist.tile([P, QBLK], f32)              # 1 / row sums
        warm = persist.tile([P, 1], f32)
        wdum = persist.tile([P, P], bf16)              # warm-up weights
        rdum = persist.tile([P, 512], bf16)            # warm-up rhs

        nc.vector.memset(warm[:], 0.0)
        nc.vector.memset(wdum[:], 0.0)
        nc.vector.memset(rdum[:], 0.0)

        # ---- input DMAs: pair0 + queries get the engines first; the
        # first query block consumes chunks in this arrival order ----
        nc.scalar.dma_start(x8_sb[:, 0], x8_r[:, 0])
        nc.sync.dma_start(xq_sb[:], xq_r[:])
        nc.gpsimd.dma_start(x8_sb[:, 2], x8_r[:, 2])
        nc.scalar.activation(warm[:], warm[:], AF.Exp)  # ACT table load now
        nc.scalar.dma_start(x8_sb[:, 1], x8_r[:, 1])
        nc.scalar.dma_start(x8_sb[:, 3], x8_r[:, 3])

        # ---- PE pstate warm-up: keep the Tensor engine busy through
        # the input-DMA window so real matmuls run at full clock and
        # overlap their LDWEIGHTS from the first real instruction ----
        for _ in range(_NWARM):
            pd = psum.tile([P, 4, 512], f32, tag="pp", name="pp", bufs=2)
            nc.tensor.matmul(pd[:, 0, :], lhsT=wdum[:], rhs=rdum[:],
                             start=True, stop=True)

        # ---- head: blocks 0 and 1 run chunk-granular so exps follow
        # the DMA arrivals; while pair1 is in flight, ACT fills the
        # wait with block 1's pair-0 chunk (pair0 serves every block)
        rq0 = persist.tile([P, 4], f32)
        rq1 = persist.tile([P, 3], f32)

        def chunk_block(qb, c, acc):
            pp = psum.tile([P, 4, 512], f32, tag="pp", name="pp", bufs=2)
            qsl = slice(qb * P, qb * P + P)
            for j in range(2):
                cs = slice(j * 512, j * 512 + 512)
                for k2 in range(2):
                    k2s = slice(2 * k2, 2 * k2 + 2)
                    nc.tensor.matmul(
                        pp[:, j, :],
                        lhsT=xq_sb[:, k2s, qsl],
                        rhs=x8_sb[:, c, k2s, cs],
                        start=(k2 == 0),
                        stop=(k2 == 1),
                        perf_mode=DR,
                    )
            eo = slice((c % 2) * 1024, (c % 2) * 1024 + 1024)
            nc.scalar.activation(
                e_sb[:, qb, c // 2, eo].rearrange("p (a x) -> p a x", a=2),
                pp[:, 0:2, :], AF.Exp, scale=ESC, accum_out=acc)

        def half1_block(qb, acc):
            # one wide exp over keys 2048:4096 (pairs 2+3)
            pp = psum.tile([P, 4, 512], f32, tag="pp", name="pp", bufs=2)
            qsl = slice(qb * P, qb * P + P)
            for j in range(4):
                cs = slice((j % 2) * 512, (j % 2) * 512 + 512)
                for k2 in range(2):
                    k2s = slice(2 * k2, 2 * k2 + 2)
                    nc.tensor.matmul(
                        pp[:, j, :],
                        lhsT=xq_sb[:, k2s, qsl],
                        rhs=x8_sb[:, 2 + j // 2, k2s, cs],
                        start=(k2 == 0),
                        stop=(k2 == 1),
                        perf_mode=DR,
                    )
            nc.scalar.activation(
                e_sb[:, qb, 1].rearrange("p (a x) -> p a x", a=4), pp[:],
                AF.Exp, scale=ESC, accum_out=acc)

        rq2 = persist.tile([P, 3], f32)
        rq3 = persist.tile([P, 3], f32)
        chunk_block(0, 0, rq0[:, 0:1])
        chunk_block(1, 0, rq1[:, 0:1])   # fillers while pair1 lands
        chunk_block(2, 0, rq2[:, 0:1])
        chunk_block(0, 1, rq0[:, 1:2])
        chunk_block(1, 1, rq1[:, 1:2])
        chunk_block(2, 1, rq2[:, 1:2])
        chunk_block(0, 2, rq0[:, 2:3])
        chunk_block(3, 0, rq3[:, 0:1])   # filler while pair3 lands
        chunk_block(0, 3, rq0[:, 3:4])
        chunk_block(3, 1, rq3[:, 1:2])
        nc.vector.tensor_reduce(rs[:, 0:1], rq0[:],
                                axis=mybir.AxisListType.X,
                                op=mybir.AluOpType.add)
        nc.vector.reciprocal_approx_fast(rr[:, 0:1], rs[:, 0:1])
        for h in range(2):
            nc.vector.tensor_scalar_mul(e_sb[:, 0, h], e_sb[:, 0, h],
                                        rr[:, 0:1])
            nc.sync.dma_start(out_r[:, 0, 4 * h:4 * h + 4],
                              e_sb[:, 0, h].rearrange(
                                  "p (nk x) -> p nk x", x=512))
        for qb, rqt in ((1, rq1), (2, rq2)):
            half1_block(qb, rqt[:, 2:3])
            nc.vector.tensor_reduce(rs[:, qb:qb + 1], rqt[:],
                                    axis=mybir.AxisListType.X,
                                    op=mybir.AluOpType.add)
            nc.vector.reciprocal_approx_fast(rr[:, qb:qb + 1],
                                             rs[:, qb:qb + 1])
            nc.vector.tensor_scalar_mul(e_sb[:, qb], e_sb[:, qb],
                                        rr[:, qb:qb + 1])
            nc.gpsimd.dma_start(out_r[:, qb], e_sb[:, qb].rearrange(
                "p h (nk x) -> p (h nk) x", x=512))

        # ---- last block: h0 was consumed as head fillers; one wide
        # exp for h1, then split scale + DMA across both free queues
        half1_block(3, rq3[:, 2:3])
        nc.vector.tensor_reduce(rs[:, 3:4], rq3[:],
                                axis=mybir.AxisListType.X,
                                op=mybir.AluOpType.add)
        nc.vector.reciprocal_approx_fast(rr[:, 3:4], rs[:, 3:4])
        for h in range(2):
            nc.vector.tensor_scalar_mul(e_sb[:, 3, h], e_sb[:, 3, h],
                                        rr[:, 3:4])
            eng = nc.sync if h == 0 else nc.scalar
            eng.dma_start(out_r[:, 3, 4 * h:4 * h + 4],
                          e_sb[:, 3, h].rearrange(
                              "p (nk x) -> p nk x", x=512))

    nc.compile()
    return nc


def kernel(**inputs) -> np.ndarray:
    global _built, LAST_RESULT
    import ml_dtypes

    x = np.asarray(inputs["x"], dtype=np.float32)
    C, W, H = x.shape
    N = W * H
    QB = N // _NCORES
    x2 = x.reshape(C, N)

    if _built is None or _built[1:] != (C, N):
        _built = (_build(C, N, QB), C, N)
    nc = _built[0]

    from concourse import bass_utils

    # host preprocess: unit-normalize columns, fp8-quantize, and permute
    # into the device's per-partition layout (4 KB DMA runs).
    norms = np.sqrt((x2 * x2).sum(axis=0))
    x8 = (x2 * (_CQ / norms)[None, :]).astype(ml_dtypes.float8_e4m3fn)
    # x8[ko*128+p, c*1024+j] -> x8p[p, c, ko, j]
    x8p = np.ascontiguousarray(
        x8.reshape(C // _P, _P, N // 1024, 1024).transpose(1, 2, 0, 3)
    ).reshape(_P, -1)
    in_maps = []
    for i in range(_NCORES):
        xq = x8[:, i * QB:(i + 1) * QB]
        # xq[ko*128+p, q] -> xqp[p, ko, q]
        xqp = np.ascontiguousarray(
            xq.reshape(C // _P, _P, QB).transpose(1, 0, 2)).reshape(_P, -1)
        in_maps.append({"x8": x8p, "xq": xqp})

    kwargs = {}
    if TRACE:
        kwargs["trace"] = True
        if TRACE_CORES is not None:
            kwargs["trace_cores"] = list(TRACE_CORES)
    res = bass_utils.run_bass_kernel_spmd(
        nc, in_maps, core_ids=list(range(_NCORES)), **kwargs
    )
    LAST_RESULT = res
    out = np.empty((N, N), dtype=np.float32)
    for i in range(_NCORES):
        out[i * QB:(i + 1) * QB] = res.results[i]["out"].astype(np.float32)
    return out.reshape(1, N, N)
